# revision 42
# baseline (speedup 1.0000x reference)
"""Trainium2 Bass kernel for the SNN Net (antenna-fuse -> hidden -> LIF scan
-> time-fuse -> output -> softmax), data-parallel over 8 NeuronCores.

Self-contained: hardcodes shapes/sharding; builds the Bass/Tile program and
runs it via run_bass_kernel_spmd.

v5 design notes (per core, bs=256 rows = 2 chunks of 128 partitions):
- The dominant cost on the grading runtime is per-DMA-work-item overhead
  (baseline: 360 x-loads x 128 row-descriptors of 4 KB = 46,080 descriptors
  ~= 78.8 ms at ~1.7 us each; equally consistent with ~365 dma_starts at
  ~216 us each). Descriptors are bytes / innermost-contiguous-run, and BIR
  AP dims cap runs at 16383 elems (~60 KB), so the descriptor floor is
  94.4 MB / 60 KB ~= 1536 for ANY tiling >= 15 timesteps. This version sits
  at that floor AND minimizes dma_starts: x streams in 4 single-buffered
  loads of [128, 45*4096B] (3x60 KB runs/partition -> 384 descriptors each),
  plus one [20,640] const load (20 descriptors) and one [4,128] output store
  (4): 6 dma_starts, ~1560 descriptors. x loads alternate the SP/ACT HWDGE
  rings.
- antenna fuse: qA = x[aA]*rA + x[pA], qB = x[aB]*rB + x[pB] on DVE,
  qB *= cc in-place on ACT, fused = qB + qA on GpSimd; hidden matmul via PE
  transpose (fp32) of fused into PSUM, ACT copy to SBUF, then 2 accumulating
  matmuls per timestep against wpp [128d, 10h].
- sn for each 45-t group lives in two PSUM banks (half-group split, both
  row-chunks together); each bank is pre-biased with ONE K=1 ones-row matmul
  (start=True sets has_written everywhere; per-t matmuls accumulate onto it)
  so the LIF scan needs one u-op per step and starts mid-group; spikes are
  scaled by w_time[t] via tensor_scalar immediates; the time-fuse sum is
  folded per group (no big spk buffer or end reduce).
- head: output linear + softmax on DVE/ACT; result is PE-transposed to
  [4,128] so the output DMA is 4 contiguous descriptors.
- TimelineSim: 457 us/core, 2078 instructions (single-buffered x trades
  ~150 us of device-side overlap for fewer dma_starts; a 2-buffered
  groups=[15]*6 build (with split_edges=True) sims at ~294 us if device time ever matters more than
  DMA work-item count). Verified bit-stable vs jax reference also for
  adversarial weights (zero antenna weight, negated w_time).
"""

import os
import sys
from contextlib import ExitStack

import numpy as np

for _p in ("/opt/trn_rl_repo", "/root/.axon_site/_ro/trn_rl_repo"):
    if _p not in sys.path and os.path.isdir(_p):
        sys.path.insert(0, _p)

import concourse.bacc as bacc
import concourse.bass as bass
import concourse.mybir as mybir
import concourse.tile as tile
from concourse.bass_utils import run_bass_kernel_spmd

F32 = mybir.dt.float32
ALU = mybir.AluOpType

B, T, A, D, H, O = 2048, 90, 4, 256, 10, 2
N_CORES = 8
BS = B // N_CORES          # 256 batch rows per core
NB = BS // 128             # 2 row-chunks of 128 partitions
CHW = NB * H               # 20 free elems per scan step
BETA = 0.95
THR = 1.0
TG = 15                    # default timesteps per x DMA (60 KB per partition
                           # row; 15360 elems stays one <=16383-elem AP run)
SG = 2                     # timesteps per antenna-fuse DVE op / PSUM pair


def _pick_pairs(w_ant):
    """Order the 4 antennas into two (pivot, other) pairs so the global
    max-|w| antenna is the pivot of pair A. Returns indices and folded
    scalars (rA, rB, cc, base) with |rA|,|rB|,|cc| <= 1 and base = w[pA]."""
    w = np.asarray(w_ant, np.float64)
    order = np.argsort(-np.abs(w))
    pA, aA = int(order[0]), int(order[3])
    pB, aB = int(order[1]), int(order[2])
    base = float(w[pA])

    def safe_div(n, d):
        return float(n / d) if abs(d) > 0.0 else 0.0

    rA = safe_div(w[aA], w[pA])
    rB = safe_div(w[aB], w[pB])
    cc = safe_div(w[pB], w[pA])
    return (pA, aA, pB, aB), (rA, rB, cc, base)


def _build(sc, bs=BS, t_steps=T, qp_bufs=2, ftp_bufs=3, groups=None,
           xp_bufs=2, split_edges=False, spk_bufs=2):
    """Emit the Bass program. sc: dict of host-folded scalars/lists.
    groups: list of per-DMA timestep counts (sums to t_steps)."""
    rA, rB, cc = sc["rA"], sc["rB"], sc["cc"]
    pA, aA, pB, aB = sc["idx"]
    w_time = sc["w_time"]          # list of 90 floats (immediates)
    b_time = sc["b_time"]
    b_out = sc["b_out"]

    nb = bs // 128
    assert bs % 128 == 0 and nb == 2, "kernel assumes 256 rows/core"
    if groups is None:
        groups = [TG] * (t_steps // TG)
    assert sum(groups) == t_steps
    gmax = max(groups)

    nc = bacc.Bacc()
    x_d = nc.dram_tensor("x", (bs, t_steps * A * D), F32, kind="ExternalInput")
    wc_d = nc.dram_tensor("wc", (CHW, 640), F32, kind="ExternalInput")
    out_d = nc.dram_tensor("out", (nb * O, 128), F32, kind="ExternalOutput")

    with ExitStack() as ctx:
        tc = ctx.enter_context(tile.TileContext(nc))
        consts = ctx.enter_context(tc.tile_pool(name="consts", bufs=1))
        xp = ctx.enter_context(tc.tile_pool(name="xp", bufs=xp_bufs))
        qp = ctx.enter_context(tc.tile_pool(name="qp", bufs=qp_bufs))
        ftp = ctx.enter_context(tc.tile_pool(name="ftp", bufs=ftp_bufs))
        state = ctx.enter_context(tc.tile_pool(name="state", bufs=2))
        spkp = ctx.enter_context(tc.tile_pool(name="spk", bufs=spk_bufs))
        outp = ctx.enter_context(tc.tile_pool(name="outp", bufs=1))
        ps_ft = ctx.enter_context(tc.tile_pool(name="ps_ft", bufs=3, space="PSUM"))
        ps_sn = ctx.enter_context(tc.tile_pool(name="ps_sn", bufs=2, space="PSUM"))
        ps_ms = ctx.enter_context(tc.tile_pool(name="ps_ms", bufs=1, space="PSUM"))

        # ---- constants ----
        ident = consts.tile([128, 128], F32)
        from concourse.masks import make_identity
        make_identity(nc, ident)

        ones1 = consts.tile([1, 128], F32)
        nc.vector.memset(ones1, 1.0)

        wc = consts.tile([CHW, 640], F32)
        nc.sync.dma_start(out=wc, in_=wc_d[:, :])
        wppT = wc[:, 0:128]
        bcomb = wc[0:1, 128:128 + H]
        bcomb_rep = wc[0:1, 180:180 + 460]   # b_comb tiled; slice per bank
        # wpp[p, k*H+h] = w_hid[h, k*128+p] * base   (PE transpose of wppT)
        wpp_ps = ps_ms.tile([128, CHW], F32, tag="misc")
        nc.tensor.matmul(wpp_ps, lhsT=wppT, rhs=ident[0:CHW, 0:CHW],
                         is_transpose=True, start=True, stop=True,
                         skip_group_check=True)
        wpp = consts.tile([128, CHW], F32)
        nc.scalar.copy(out=wpp, in_=wpp_ps)

        # broadcast w_out row to all partitions: [128, O*CHW]
        woutb_ps = ps_ms.tile([128, O * CHW], F32, tag="misc")
        nc.tensor.matmul(woutb_ps, lhsT=ones1, rhs=wc[0:1, 128 + H:128 + H + O * CHW],
                         start=True, stop=True, skip_group_check=True)
        woutb = consts.tile([128, O * CHW], F32)
        nc.scalar.copy(out=woutb, in_=woutb_ps)

        mem = state.tile([128, nb, H], F32, tag="mem")
        nc.vector.memset(mem, 0.0)
        ftacc = outp.tile([128, nb, H], F32)
        nc.vector.memset(ftacc, 0.0)

        tbase = 0
        for g, gs in enumerate(groups):
            th = (gs + 1) // 2     # first-half timesteps per sn bank
            # two PSUM banks hold sn for BOTH row-chunks of this group,
            # split by half-group so the scan can start mid-group
            snA = ps_sn.tile([128, nb, th, H], F32, tag="snA")
            snB = ps_sn.tile([128, nb, gs - th, H], F32, tag="snB")
            # pre-bias each bank in one K=1 matmul (start=True sets
            # has_written for every element; per-t matmuls accumulate)
            nc.tensor.matmul(snA[:], lhsT=ones1,
                             rhs=bcomb_rep[:, 0:nb * th * H],
                             start=True, stop=False, skip_group_check=True)
            nc.tensor.matmul(snB[:], lhsT=ones1,
                             rhs=bcomb_rep[:, 0:nb * (gs - th) * H],
                             start=True, stop=False, skip_group_check=True)
            for c in range(nb):
                x_flat = xp.tile([128, gmax * A * D], F32, tag="x")
                src = x_d[c * 128:(c + 1) * 128,
                          tbase * A * D:(tbase + gs) * A * D]
                # alternate the two HWDGE rings (SP / ACT) for the x stream
                dma_eng = nc.sync if (g * nb + c) % 2 == 0 else nc.scalar
                if split_edges and (g == 0 or
                                    (g == len(groups) - 1 and c == nb - 1)):
                    cuts = [0, (gs // 2 + 1) * A * D, gs * A * D]
                else:
                    cuts = [0, gs * A * D]
                for lo, hi in zip(cuts[:-1], cuts[1:]):
                    dma_eng.dma_start(out=x_flat[:, lo:hi], in_=src[:, lo:hi])
                x_t = x_flat[:, 0:gs * A * D].rearrange(
                    "p (t a d) -> p t a d", t=gs, a=A, d=D)

                for t0 in range(0, gs, SG):
                    sg = min(SG, gs - t0)
                    qA_t = qp.tile([128, SG, D], F32, tag="qA")
                    qA = qA_t[:, 0:sg]
                    nc.vector.scalar_tensor_tensor(
                        out=qA, in0=x_t[:, t0:t0 + sg, aA], scalar=rA,
                        in1=x_t[:, t0:t0 + sg, pA], op0=ALU.mult, op1=ALU.add)
                    qB_t = qp.tile([128, SG, D], F32, tag="qB")
                    qB = qB_t[:, 0:sg]
                    nc.vector.scalar_tensor_tensor(
                        out=qB, in0=x_t[:, t0:t0 + sg, aB], scalar=rB,
                        in1=x_t[:, t0:t0 + sg, pB], op0=ALU.mult, op1=ALU.add)
                    nc.scalar.mul(qB, qB, cc)     # in-place scale on ACT
                    fused_t = qp.tile([128, SG, D], F32, tag="fused")
                    fused = fused_t[:, 0:sg]
                    nc.gpsimd.tensor_tensor(out=fused, in0=qB, in1=qA,
                                            op=ALU.add)
                    # transpose the sg timesteps (2 halves each) into PSUM
                    ftps = ps_ft.tile([128, SG * D], F32, tag="ftps")
                    for tl in range(sg):
                        for h2 in range(2):
                            nc.tensor.matmul(
                                ftps[:, (tl * 2 + h2) * 128:(tl * 2 + h2 + 1) * 128],
                                lhsT=fused[:, tl, h2 * 128:(h2 + 1) * 128],
                                rhs=ident, is_transpose=True,
                                start=True, stop=True, skip_group_check=True)
                    fT = ftp.tile([128, SG * D], F32, tag="fT")
                    nc.scalar.copy(out=fT[:, 0:sg * D], in_=ftps[:, 0:sg * D])
                    for tl in range(sg):
                        tt = t0 + tl
                        sl = (snA[:, c, tt] if tt < th
                              else snB[:, c, tt - th])
                        for h2 in range(2):
                            nc.tensor.matmul(
                                sl,
                                lhsT=fT[:, (tl * 2 + h2) * 128:(tl * 2 + h2 + 1) * 128],
                                rhs=wpp[:, h2 * H:(h2 + 1) * H],
                                start=False, stop=(h2 == 1),
                                skip_group_check=True)
            # ---- LIF scan over this group's timesteps ----
            spk_g = spkp.tile([128, gmax, nb, H], F32, tag="spkg")
            for tl in range(gs):
                t = tbase + tl
                u = state.tile([128, nb, H], F32, tag="u")
                inp = snA[:, :, tl] if tl < th else snB[:, :, tl - th]
                mem_new = state.tile([128, nb, H], F32, tag="mem")
                nc.vector.scalar_tensor_tensor(
                    out=u, in0=mem, scalar=BETA, in1=inp,
                    op0=ALU.mult, op1=ALU.add)
                nc.vector.scalar_tensor_tensor(
                    out=mem_new, in0=mem, scalar=THR, in1=u,
                    op0=ALU.is_le, op1=ALU.mult)
                nc.vector.tensor_scalar(
                    out=spk_g[:, tl], in0=mem_new,
                    scalar1=THR, scalar2=w_time[t],
                    op0=ALU.is_gt, op1=ALU.mult)
                mem = mem_new
            # fold this group's weighted spikes into the running time-fuse sum
            red = state.tile([128, nb, H], F32, tag="red")
            nc.vector.tensor_reduce(
                out=red, in_=spk_g[:, 0:gs].rearrange("p t c h -> p c h t"),
                axis=mybir.AxisListType.X, op=ALU.add)
            nc.vector.tensor_tensor(out=ftacc, in0=ftacc, in1=red, op=ALU.add)
            tbase += gs

        # ---- output head + softmax ----
        ft = outp.tile([128, CHW], F32)
        nc.vector.tensor_scalar_add(
            out=ft, in0=ftacc[:].rearrange("p c h -> p (c h)"), scalar1=b_time)
        lg = outp.tile([128, O * nb], F32)          # cols o*nb + c
        for o in range(O):
            mo = outp.tile([128, CHW], F32, tag="mo")
            nc.vector.tensor_tensor(out=mo, in0=ft,
                                    in1=woutb[:, o * CHW:(o + 1) * CHW],
                                    op=ALU.mult)
            nc.vector.tensor_reduce(
                out=lg[:, o * nb:(o + 1) * nb],
                in_=mo[:].rearrange("p (c h) -> p c h", h=H),
                axis=mybir.AxisListType.X, op=ALU.add)
            nc.vector.tensor_scalar_add(
                out=lg[:, o * nb:(o + 1) * nb],
                in0=lg[:, o * nb:(o + 1) * nb], scalar1=b_out[o])
        ex = outp.tile([128, O * nb], F32)
        nc.scalar.activation(out=ex, in_=lg,
                             func=mybir.ActivationFunctionType.Exp)
        ssum = outp.tile([128, nb], F32)
        nc.vector.tensor_tensor(out=ssum, in0=ex[:, 0:nb],
                                in1=ex[:, nb:2 * nb], op=ALU.add)
        rec = outp.tile([128, nb], F32)
        nc.vector.reciprocal(out=rec, in_=ssum)
        res = outp.tile([128, nb * O], F32)         # cols c*O + o
        for c in range(nb):
            for o in range(O):
                nc.vector.tensor_tensor(
                    out=res[:, c * O + o: c * O + o + 1],
                    in0=ex[:, o * nb + c: o * nb + c + 1],
                    in1=rec[:, c: c + 1], op=ALU.mult)
        # transpose to [4, 128] so the output DMA is 4 contiguous rows
        resT_ps = ps_ms.tile([nb * O, 128], F32, tag="misc")
        nc.tensor.matmul(resT_ps, lhsT=res, rhs=ident, is_transpose=True,
                         start=True, stop=True, skip_group_check=True)
        resT = outp.tile([nb * O, 128], F32)
        nc.scalar.copy(out=resT, in_=resT_ps)
        nc.sync.dma_start(out=out_d[:, :], in_=resT)
    nc.finalize()
    return nc


def _prep_weights(w_ant, b_ant, w_hid, b_hid, w_time, b_time, w_out, b_out):
    """Host-side weight folding. Returns (scalars, const_arrays)."""
    w_ant = np.asarray(w_ant, np.float32)
    w_hid = np.asarray(w_hid, np.float32)
    w_out = np.asarray(w_out, np.float32)
    idx, (rA, rB, cc, base) = _pick_pairs(w_ant)
    # wppT[k*H+h, p] = w_hid[h, k*128+p] * base
    wppT = np.empty((CHW, 128), np.float32)
    for k in range(NB):
        wppT[k * H:(k + 1) * H, :] = (w_hid[:, k * 128:(k + 1) * 128]
                                      * np.float32(base))
    b_comb = (np.float32(b_ant) * w_hid.sum(axis=1)
              + np.asarray(b_hid, np.float32)).astype(np.float32)
    wc = np.zeros((CHW, 640), np.float32)
    wc[:, 0:128] = wppT
    wc[0, 128:128 + H] = b_comb
    # wout row: [o*CHW + c*H + h] = w_out[o, h]
    wc[0, 128 + H:128 + H + O * CHW] = np.concatenate(
        [np.tile(w_out[o], NB) for o in range(O)])
    wc[0, 180:180 + 460] = np.tile(b_comb, 46)
    scalars = {"rA": rA, "rB": rB, "cc": cc, "idx": idx,
               "w_time": [float(v) for v in np.asarray(w_time, np.float32)],
               "b_time": float(np.float32(b_time)),
               "b_out": [float(v) for v in np.asarray(b_out, np.float32)]}
    consts = {"wc": wc}
    return scalars, consts


_CACHE = {}


def kernel(x, w_ant, b_ant, w_hid, b_hid, w_time, b_time, w_out, b_out):
    x = np.ascontiguousarray(np.asarray(x, np.float32))
    assert x.shape == (B, T, A, D), x.shape
    scalars, consts = _prep_weights(w_ant, b_ant, w_hid, b_hid, w_time,
                                    b_time, w_out, b_out)
    key = (scalars["rA"], scalars["rB"], scalars["cc"], scalars["idx"],
           tuple(scalars["w_time"]), scalars["b_time"],
           tuple(scalars["b_out"]))
    nc = _CACHE.get(key)
    if nc is None:
        nc = _build(scalars, BS, T, groups=[45, 45], xp_bufs=1,
                    qp_bufs=2, ftp_bufs=2, spk_bufs=1)
        _CACHE[key] = nc
    in_maps = []
    for i in range(N_CORES):
        xs = np.ascontiguousarray(x[i * BS:(i + 1) * BS]).reshape(BS, T * A * D)
        m = {"x": xs}
        m.update(consts)
        in_maps.append(m)
    r = run_bass_kernel_spmd(nc, in_maps, core_ids=list(range(N_CORES)))
    out = np.empty((B, O), np.float32)
    for i in range(N_CORES):
        arr = r.results[i]["out"]          # [nb*O, 128], rows c*O + o
        for c in range(NB):
            blk = arr[c * O:(c + 1) * O, :]            # [O, 128]
            out[i * BS + c * 128:i * BS + (c + 1) * 128, :] = blk.T
    return out
